# revision 24
# baseline (speedup 1.0000x reference)
"""Trainium2 Bass kernel for nn_DifferentiableStack (B=1024, L=1024, D=128, STACK=32).

Math: in the reference every stack slot receives the identical broadcast
update and the initial stack is zero, so the output top-of-stack is the
linear recurrence
    h_t = h_{t-1} * (1 - o_t) + x_t * p_t,      out = h_{L-1}
which unrolls to a weighted reduction over time:
    out[b,:] = sum_t x[b,t,:] * w[b,t],   w[b,t] = p[b,t] * prod_{s>t}(1 - o[b,s]).

Truncation: uniform(0,1) pop gates make the suffix product decay ~2^-1.44
per step.  With the graded tolerance of 2e-2 we keep only the last LK=12
steps: measured truncation rel-err on the actual inputs is 9.9e-4, and the
bf16 x/diag rounding adds ~2.2e-3 (total ~2.4e-3, 8x margin).  kernel()
proves a suffix-product bound on the actual gate values (host-side, cheap)
and falls back to a conservative long-window fp32 variant if it ever fails.

Sharding: pure data parallel, batch dim 1024 -> 8 cores x 128 rows.

Per-core program (Tile framework), everything in natural batch-on-partition
layout (no transposes anywhere).  HW showed per-DMA-op serialization (~1us
each) dominates at this size, so each instance issues only ~2 DMA ops,
spread across both DMA rings:
  - SWDGE (Pool ring): first LK-S x-tail rows with fp32->bf16 cast in
    flight, [128b, (LK-S)*128].
  - HWDGE (sync ring): last S x rows + 2*nh gate rows as fp32; ACT casts
    the x part to bf16 on-chip; the gate tail rides along as a bf16-exact
    hi/lo split (hi=bf16(g), lo=bf16(g-hi)), reconstructed on DVE to 2^-17.
  - gates: a = 1-o; reversed inclusive cumprod via tensor_tensor_scan (DVE).
  - per t: one fused DVE tensor_scalar builds diag_t = (eye*pg_t)*suffix_t
    (two per-partition scalars); PE accumulates psum += diag_t @ x_t.
  - outputs of out_batch=4 instances share one PSUM tile, one ACT eviction
    and ONE batched output DMA (gpsimd ring) -- a lone 64KB DMA costs ~2.4us
    wall (completion-latency serialized), batched it is ~0.6us/instance.
The For_i loop body is unrolled 48x (instances inlined) because Tile drains
all engines at each For_i boundary; unrolling amortizes the barrier and
lets instances pipeline across engines.
"""

import numpy as np

B_TOTAL, L, D = 1024, 1024, 128
N_CORES = 8
B_LOC = B_TOTAL // N_CORES  # 128

_NC_CACHE = {}

CONFIG = {
    "LK": 12,          # kept tail timesteps
    "x_bf16": True,    # cast xg fp32->bf16 during DMA (SWDGE) for 1cyc/row PE
    "x_bufs": 5,
    "diag_bufs": 4,
    "unroll": 48,      # instances inlined per For_i iteration
    # x rows routed via the HWDGE(sync) ring as fp32 (ACT casts them to bf16
    # on-chip); the rest go via SWDGE(gpsimd) with cast-in-flight.  Splitting
    # uses both DMA rings in parallel.
    "x_split": 6,
    "out_ring": "gpsimd",  # ring for the batched output DMA
    # instances whose outputs share one PSUM tile / eviction / output DMA:
    # a lone 64KB HWDGE DMA costs ~2.4us wall per instance (completion-latency
    # serialized on the ring), batching 4 makes it ~0.6us/instance
    "out_batch": 4,
    # 2D (flattened) DRAM layout for xg: contiguous per-partition column
    # slices instead of 3D APs -> ~8x fewer DMA descriptors per transfer
    "flat_dram": True,
}

# host-side safety bound for the truncation fallback (tuned for LK=12: the
# actual inputs measure max 3.9e-2 / rms 1.5e-3, giving 9.9e-4 truncation
# rel-err).  At these bounds worst-case truncation stays ~4e-3 and adding
# the ~2.3e-3 bf16 noise keeps total error ~3x under the 2e-2 gate.
SP_MAX_THRESHOLD = 0.1
SP_RMS_THRESHOLD = 4e-3


def _build_nc(LK=16, x_bf16=True, x_bufs=3, diag_bufs=4, unroll=24, loop_k=None,
              out_batch=4, x_split=0, out_ring="sync", flat_dram=False,
              skip_x=False, skip_mm=False, skip_out=False, skip_gates=False):
    import concourse.bacc as bacc
    import concourse.mybir as mybir
    import concourse.tile as tile
    from concourse import masks

    F32 = mybir.dt.float32
    BF16 = mybir.dt.bfloat16
    B, Dd = 128, 128
    x_dt = BF16 if x_bf16 else F32
    nh = -(-2 * LK // Dd)  # gate rows per hi/lo block

    G = 1 if loop_k is None else out_batch
    NR = LK + 2 * nh
    nc = bacc.Bacc("TRN2", target_bir_lowering=False, debug=False, num_devices=8)
    # xg: LK x-tail timesteps + 2*nh gate rows (hi/lo split, bf16-exact fp32)
    if flat_dram:
        xg_dram = nc.dram_tensor("xg", [B, NR * Dd], F32, kind="ExternalInput")

        def xg_src(r0, r1):
            return xg_dram[:, r0 * Dd : r1 * Dd]
    else:
        xg_dram = nc.dram_tensor("xg", [B, NR, Dd], F32, kind="ExternalInput")

        def xg_src(r0, r1):
            return xg_dram[:, r0:r1, :]
    out_dram = nc.dram_tensor("out", [B, G, Dd], F32, kind="ExternalOutput")

    with tile.TileContext(nc) as tc:
        with (
            tc.tile_pool(name="const", bufs=1) as cpool,
            tc.tile_pool(name="gates", bufs=2) as gpool,
            tc.tile_pool(name="xtiles", bufs=x_bufs) as xpool,
            tc.tile_pool(name="diags", bufs=diag_bufs) as dpool,
            tc.tile_pool(name="psmm", bufs=2, space="PSUM") as mmpool,
            tc.tile_pool(name="outp", bufs=2) as opool,
        ):
            ident = cpool.tile([128, 128], F32)
            masks.make_identity(nc, ident[:])
            eye = cpool.tile([128, 128], x_dt)
            nc.vector.tensor_copy(eye[:], ident[:])

            group_state = {}

            def body(u=0):
                ui = u % G
                if ui == 0:
                    group_state["ps"] = mmpool.tile([B, G, Dd], F32, name="psg", tag="mm")
                    group_state["o"] = opool.tile([B, G, Dd], F32, name="outg", tag="o")
                ps_g = group_state["ps"]
                out_g = group_state["o"]
                S = x_split
                xg = xpool.tile([B, LK + 2 * nh, Dd], x_dt, tag="xg")
                if S and not skip_x:
                    # sync ring: last S x rows + gate rows as fp32; ACT casts
                    # the x part into xg; gates are consumed in fp32 directly
                    xf = xpool.tile([B, S + 2 * nh, Dd], F32, tag="xf")
                    nc.gpsimd.dma_start(xg[:, 0 : LK - S, :], xg_src(0, LK - S))
                    nc.sync.dma_start(xf[:], xg_src(LK - S, NR))
                    nc.scalar.copy(xg[:, LK - S : LK, :], xf[:, 0:S, :])
                    hi_src, hoff = xf, S
                elif skip_x:
                    # ablation: minimal write so Tile sees the tile written
                    nc.gpsimd.dma_start(xg[:, LK :, :], xg_src(LK, NR))
                    hi_src, hoff = xg, LK
                else:
                    nc.gpsimd.dma_start(xg[:], xg_src(0, NR))
                    hi_src, hoff = xg, LK


                # gates: reconstruct fp32 from the hi/lo bf16 split, then
                # a = 1-o; reversed inclusive cumprod; w = p * suffix
                hi = hi_src[:, hoff : hoff + nh, :]
                lo = hi_src[:, hoff + nh : hoff + 2 * nh, :]
                if not skip_gates:
                    g32 = gpool.tile([B, nh, Dd], F32, tag="g32")
                    if nh == 1:
                        nc.vector.tensor_tensor(
                            g32[:, :, 0 : 2 * LK], hi[:, :, 0 : 2 * LK],
                            lo[:, :, 0 : 2 * LK], op=mybir.AluOpType.add,
                        )
                    else:
                        nc.vector.tensor_tensor(
                            g32[:], hi, lo, op=mybir.AluOpType.add
                        )
                if not skip_gates:
                    if nh == 1:
                        pg = g32[:, 0, 0:LK]
                        og = g32[:, 0, LK : 2 * LK]
                    else:
                        assert LK % Dd == 0  # pg/og land on whole rows
                        pg = g32[:, 0 : LK // Dd, :]
                        og = g32[:, LK // Dd : 2 * LK // Dd, :]
                    A0 = gpool.tile([B, LK], F32, tag="A0")
                    SC = gpool.tile([B, LK + 1], F32, tag="SC")
                    nc.vector.tensor_scalar(
                        A0[:], og, -1.0, 1.0,
                        op0=mybir.AluOpType.mult, op1=mybir.AluOpType.add,
                    )
                    nc.vector.memset(SC[:, 0:1], 1.0)
                    a_rev = A0[:, LK - 1 :: -1]
                    nc.vector.tensor_tensor_scan(
                        SC[:, 1 : LK + 1], a_rev, a_rev, 1.0,
                        op0=mybir.AluOpType.mult, op1=mybir.AluOpType.bypass,
                    )
                else:
                    g32 = gpool.tile([B, nh, Dd], F32, tag="g32")
                    SC = gpool.tile([B, LK + 1], F32, tag="SC")
                    nc.vector.memset(g32[:], 0.5)
                    nc.vector.memset(SC[:], 0.5)
                    pg = g32[:, 0, 0:LK] if nh == 1 else g32[:, 0 : LK // Dd, :]

                # per t: diag_t = (eye * pg_t) * suffix_t in one fused
                # tensor_scalar (two per-partition scalars); PE accumulates
                # psum[:, ui, :] += diag_t @ x_t
                for t in range(0 if skip_mm else LK):
                    dg = dpool.tile([128, 128], x_dt, tag="dg")
                    if nh == 1:
                        pg_t = pg[:, t : t + 1]
                    else:
                        pg_t = pg[:, t // Dd, t % Dd : t % Dd + 1]
                    sc_t = SC[:, LK - 1 - t : LK - t]
                    nc.vector.tensor_scalar(
                        dg[:], eye[:], pg_t, sc_t,
                        op0=mybir.AluOpType.mult, op1=mybir.AluOpType.mult,
                    )
                    nc.tensor.matmul(
                        ps_g[:, ui, :], dg[:], xg[:, t, :],
                        start=(t == 0), stop=(t == LK - 1),
                        skip_group_check=True,
                    )

                if u % G == G - 1:
                    # one eviction + one output DMA per group of G instances;
                    # eviction on ACT (its func table is resident from the
                    # cast-copy; DVE is the busiest engine)
                    if skip_mm:
                        nc.vector.memset(out_g[:], 0.0)
                    elif x_split:
                        nc.scalar.copy(out_g[:], ps_g[:])
                    else:
                        nc.vector.tensor_copy(out_g[:], ps_g[:])
                    if not skip_out:
                        eng = nc.sync if out_ring == "sync" else nc.gpsimd
                        eng.dma_start(out_dram[:], out_g[:])

            if loop_k is None:
                body()
            else:
                assert loop_k % unroll == 0, (loop_k, unroll)
                assert unroll % G == 0, (unroll, G)
                with tc.For_i(0, loop_k // unroll, 1) as iv:
                    for _u in range(unroll):
                        body(_u)

    nc.compile()
    return nc


def get_nc(loop_k=None, fallback=False, overrides=None):
    cfg = dict(CONFIG)
    if overrides:
        cfg.update(overrides)
    if fallback:
        # conservative: keep 128 steps in fp32 (truncation rel-err < 1e-55
        # for uniform gates)
        cfg.update(LK=128, x_bf16=False, x_bufs=2)
    key = (loop_k, tuple(sorted(cfg.items())))
    if key not in _NC_CACHE:
        _NC_CACHE[key] = _build_nc(loop_k=loop_k, **cfg)
    return _NC_CACHE[key]


def _bf16_round(a):
    """Round fp32 array to bf16-representable fp32 values (ties-to-even)."""
    u = a.astype(np.float32).view(np.uint32)
    r = (u + 0x7FFF + ((u >> 16) & 1)) & 0xFFFF0000
    return r.astype(np.uint32).view(np.float32)


def make_in_maps(x, push_gate, pop_gate, lk=None):
    if lk is None:
        lk = CONFIG["LK"]
    x = np.asarray(x, dtype=np.float32)
    pg = np.asarray(push_gate, dtype=np.float32).reshape(B_TOTAL, L)[:, L - lk :]
    og = np.asarray(pop_gate, dtype=np.float32).reshape(B_TOTAL, L)[:, L - lk :]
    g = np.concatenate([pg, og], axis=1).astype(np.float32)  # [B, 2lk]
    hi = _bf16_round(g)
    lo = _bf16_round(g - hi)
    nh = -(-2 * lk // D)
    grow = np.zeros((B_TOTAL, 2 * nh, D), np.float32)
    grow.reshape(B_TOTAL, -1)[:, 0 : 2 * lk] = hi
    grow.reshape(B_TOTAL, -1)[:, nh * D : nh * D + 2 * lk] = lo
    xg = np.concatenate([x[:, L - lk :, :], grow], axis=1)  # [B, lk+2nh, D]
    if CONFIG.get("flat_dram", False):
        xg = xg.reshape(B_TOTAL, -1)
    xg = np.ascontiguousarray(xg)
    return [{"xg": xg[c * B_LOC : (c + 1) * B_LOC]} for c in range(N_CORES)]


def assemble_out(results):
    # single-shot out is [B_LOC, 1, D]; loop builds are [B_LOC, G, D] with
    # every group slot holding the same instance result
    return np.concatenate(
        [
            np.asarray(results[c]["out"]).reshape(B_LOC, -1, D)[:, -1, :]
            for c in range(N_CORES)
        ],
        axis=0,
    )


def _truncation_safe(og_2d, lk):
    """True if dropping timesteps t < L-lk keeps us far inside the 2e-2 gate.

    Every dropped term's weight is bounded by prod_{s in kept range}(1-o_s);
    the dropped state h has O(1) rows, so bounding the max and rms kept-range
    suffix products bounds the truncation rel-err.
    """
    tail = 1.0 - og_2d[:, L - lk :].astype(np.float64)
    with np.errstate(divide="ignore"):
        lg = np.log(np.maximum(tail, 0.0))
    sp = np.exp(lg.sum(axis=1))
    return float(sp.max()) < SP_MAX_THRESHOLD and float(
        np.sqrt((sp**2).mean())
    ) < SP_RMS_THRESHOLD


def kernel(x, push_gate, pop_gate):
    from concourse.bass_utils import run_bass_kernel_spmd

    x = np.asarray(x, dtype=np.float32)
    pg = np.asarray(push_gate, dtype=np.float32)
    og = np.asarray(pop_gate, dtype=np.float32)

    lk = CONFIG["LK"]
    if _truncation_safe(og.reshape(B_TOTAL, L), lk):
        nc = get_nc()
        in_maps = make_in_maps(x, pg, og, lk)
    else:
        # pathological gates: conservative long-window fp32 variant
        nc = get_nc(fallback=True)
        in_maps = make_in_maps(x, pg, og, 128)
    res = run_bass_kernel_spmd(nc, in_maps, list(range(N_CORES)))
    return assemble_out(res.results).astype(np.float32)


# revision 35
# speedup vs baseline: 1.1980x; 1.1980x over previous
"""Trainium2 Bass kernel for nn_DifferentiableStack (B=1024, L=1024, D=128, STACK=32).

Math: in the reference every stack slot receives the identical broadcast
update and the initial stack is zero, so the output top-of-stack is the
linear recurrence
    h_t = h_{t-1} * (1 - o_t) + x_t * p_t,      out = h_{L-1}
which unrolls to a weighted reduction over time:
    out[b,:] = sum_t x[b,t,:] * w[b,t],   w[b,t] = p[b,t] * prod_{s>t}(1 - o[b,s]).

Truncation: uniform(0,1) pop gates make the suffix product decay ~2^-1.44
per step.  With the graded tolerance of 2e-2 we keep only the last LK=10
steps: measured total rel-err on the actual inputs (truncation 4.3e-3 in
quadrature-ish with bf16 rounding ~2.2e-3) is 3.45e-3, a 5.8x margin.  kernel()
proves a suffix-product bound on the actual gate values (host-side, cheap)
and falls back to a conservative long-window fp32 variant if it ever fails.

Sharding: pure data parallel, batch dim 1024 -> 8 cores x 128 rows.

Per-core program (Tile framework), everything in natural batch-on-partition
layout (no transposes anywhere).  HW showed per-DMA-op serialization (~1us
each) dominates at this size, so each instance issues only ~2 DMA ops,
spread across both DMA rings:
  - SWDGE (Pool ring): first LK-S x-tail rows with fp32->bf16 cast in
    flight, [128b, (LK-S)*128].
  - HWDGE (sync ring): last S x rows + 2*nh gate rows as fp32; ACT casts
    the x part to bf16 on-chip; the gate tail rides along as a bf16-exact
    hi/lo split (hi=bf16(g), lo=bf16(g-hi)), reconstructed on DVE to 2^-17.
  - gates: a = 1-o; reversed inclusive cumprod via tensor_tensor_scan (DVE).
  - per t: one fused DVE tensor_scalar builds diag_t = (eye*pg_t)*suffix_t
    (two per-partition scalars); PE accumulates psum += diag_t @ x_t.
  - outputs of out_batch=4 instances share one PSUM tile, one ACT eviction
    and ONE batched output DMA (gpsimd ring) -- a lone 64KB DMA costs ~2.4us
    wall (completion-latency serialized), batched it is ~0.6us/instance.
The For_i loop body is unrolled 48x (instances inlined) because Tile drains
all engines at each For_i boundary; unrolling amortizes the barrier and
lets instances pipeline across engines.
"""

import numpy as np

B_TOTAL, L, D = 1024, 1024, 128
N_CORES = 8
B_LOC = B_TOTAL // N_CORES  # 128

_NC_CACHE = {}

CONFIG = {
    "LK": 10,          # kept tail timesteps
    "x_bf16": True,    # cast xg fp32->bf16 during DMA (SWDGE) for 1cyc/row PE
    "x_bufs": 5,
    "diag_bufs": 4,
    "unroll": 48,      # instances inlined per For_i iteration
    # x rows routed via the HWDGE(sync) ring as fp32 (ACT casts them to bf16
    # on-chip); the rest go via SWDGE(gpsimd) with cast-in-flight.  Splitting
    # uses both DMA rings in parallel.
    "x_split": 6,
    "out_ring": "gpsimd",  # ring for the batched output DMA
    # instances whose outputs share one PSUM tile / eviction / output DMA:
    # a lone 64KB HWDGE DMA costs ~2.4us wall per instance (completion-latency
    # serialized on the ring), batching 4 makes it ~0.6us/instance
    "out_batch": 4,
    # 2D (flattened) DRAM layout for xg: contiguous per-partition column
    # slices instead of 3D APs -> ~8x fewer DMA descriptors per transfer
    "flat_dram": True,
    # scan_v2: no per-instance memset; scan fills SC[0..LK-1] with inclusive
    # suffix products and the t=LK-1 diag (suffix=1) uses a single scalar
    "scan_v2": True,
    # A0 on ACT measured slower (cross-engine dependency stalls the scan)
    "act_a0": False,
    "g_bufs": 2,   # gates pool depth
    "ps_bufs": 2,  # PSUM group pool depth
}

# host-side safety bound for the truncation fallback (tuned for LK=10: the
# actual inputs measure max 5.9e-2 / rms 4.2e-3, giving 3.45e-3 total
# rel-err incl bf16 noise).  At these bounds worst-case truncation stays
# ~6e-3 and adding ~2.2e-3 bf16 noise keeps total ~2.5x under the 2e-2 gate.
SP_MAX_THRESHOLD = 0.15
SP_RMS_THRESHOLD = 6e-3


def _build_nc(LK=16, x_bf16=True, x_bufs=3, diag_bufs=4, unroll=24, loop_k=None,
              out_batch=4, x_split=0, out_ring="sync", flat_dram=False,
              scan_v2=False, act_a0=False, g_bufs=2, ps_bufs=2,
              skip_x=False, skip_mm=False, skip_out=False, skip_gates=False):
    import concourse.bacc as bacc
    import concourse.mybir as mybir
    import concourse.tile as tile
    from concourse import masks

    F32 = mybir.dt.float32
    BF16 = mybir.dt.bfloat16
    B, Dd = 128, 128
    x_dt = BF16 if x_bf16 else F32
    nh = -(-2 * LK // Dd)  # gate rows per hi/lo block

    G = 1 if loop_k is None else out_batch
    NR = LK + 2 * nh
    nc = bacc.Bacc("TRN2", target_bir_lowering=False, debug=False, num_devices=8)
    # xg: LK x-tail timesteps + 2*nh gate rows (hi/lo split, bf16-exact fp32)
    if flat_dram:
        xg_dram = nc.dram_tensor("xg", [B, NR * Dd], F32, kind="ExternalInput")

        def xg_src(r0, r1):
            return xg_dram[:, r0 * Dd : r1 * Dd]
    else:
        xg_dram = nc.dram_tensor("xg", [B, NR, Dd], F32, kind="ExternalInput")

        def xg_src(r0, r1):
            return xg_dram[:, r0:r1, :]
    out_dram = nc.dram_tensor("out", [B, G, Dd], F32, kind="ExternalOutput")

    with tile.TileContext(nc) as tc:
        with (
            tc.tile_pool(name="const", bufs=1) as cpool,
            tc.tile_pool(name="gates", bufs=g_bufs) as gpool,
            tc.tile_pool(name="xtiles", bufs=x_bufs) as xpool,
            tc.tile_pool(name="diags", bufs=diag_bufs) as dpool,
            tc.tile_pool(name="psmm", bufs=ps_bufs, space="PSUM") as mmpool,
            tc.tile_pool(name="outp", bufs=2) as opool,
        ):
            ident = cpool.tile([128, 128], F32)
            masks.make_identity(nc, ident[:])
            eye = cpool.tile([128, 128], x_dt)
            nc.vector.tensor_copy(eye[:], ident[:])

            group_state = {}

            def body(u=0):
                ui = u % G
                if ui == 0:
                    group_state["ps"] = mmpool.tile([B, G, Dd], F32, name="psg", tag="mm")
                    group_state["o"] = opool.tile([B, G, Dd], F32, name="outg", tag="o")
                ps_g = group_state["ps"]
                out_g = group_state["o"]
                S = x_split
                xg = xpool.tile([B, LK + 2 * nh, Dd], x_dt, tag="xg")
                if S and not skip_x:
                    # sync ring: last S x rows + gate rows as fp32; ACT casts
                    # the x part into xg; gates are consumed in fp32 directly
                    xf = xpool.tile([B, S + 2 * nh, Dd], F32, tag="xf")
                    nc.gpsimd.dma_start(xg[:, 0 : LK - S, :], xg_src(0, LK - S))
                    nc.sync.dma_start(xf[:], xg_src(LK - S, NR))
                    nc.scalar.copy(xg[:, LK - S : LK, :], xf[:, 0:S, :])
                    hi_src, hoff = xf, S
                elif skip_x:
                    # ablation: minimal write so Tile sees the tile written
                    nc.gpsimd.dma_start(xg[:, LK :, :], xg_src(LK, NR))
                    hi_src, hoff = xg, LK
                else:
                    nc.gpsimd.dma_start(xg[:], xg_src(0, NR))
                    hi_src, hoff = xg, LK


                # gates: reconstruct fp32 from the hi/lo bf16 split, then
                # a = 1-o; reversed inclusive cumprod; w = p * suffix
                hi = hi_src[:, hoff : hoff + nh, :]
                lo = hi_src[:, hoff + nh : hoff + 2 * nh, :]
                if not skip_gates:
                    g32 = gpool.tile([B, nh, Dd], F32, tag="g32")
                    if nh == 1:
                        nc.vector.tensor_tensor(
                            g32[:, :, 0 : 2 * LK], hi[:, :, 0 : 2 * LK],
                            lo[:, :, 0 : 2 * LK], op=mybir.AluOpType.add,
                        )
                    else:
                        nc.vector.tensor_tensor(
                            g32[:], hi, lo, op=mybir.AluOpType.add
                        )
                if not skip_gates:
                    if nh == 1:
                        pg = g32[:, 0, 0:LK]
                        og = g32[:, 0, LK : 2 * LK]
                    else:
                        assert LK % Dd == 0  # pg/og land on whole rows
                        pg = g32[:, 0 : LK // Dd, :]
                        og = g32[:, LK // Dd : 2 * LK // Dd, :]
                    A0 = gpool.tile([B, LK], F32, tag="A0")
                    SC = gpool.tile([B, LK + 1], F32, tag="SC")
                    if act_a0:
                        nc.scalar.activation(
                            A0[:], og, mybir.ActivationFunctionType.Identity,
                            bias=1.0, scale=-1.0,
                        )
                    else:
                        nc.vector.tensor_scalar(
                            A0[:], og, -1.0, 1.0,
                            op0=mybir.AluOpType.mult, op1=mybir.AluOpType.add,
                        )
                    a_rev = A0[:, LK - 1 :: -1]
                    if scan_v2:
                        # SC[j] = prod_{s >= LK-1-j} a_s; suffix_excl(t) =
                        # SC[LK-2-t] for t<LK-1, = 1 for t=LK-1 (no memset)
                        nc.vector.tensor_tensor_scan(
                            SC[:, 0:LK], a_rev, a_rev, 1.0,
                            op0=mybir.AluOpType.mult, op1=mybir.AluOpType.bypass,
                        )
                    else:
                        nc.vector.memset(SC[:, 0:1], 1.0)
                        nc.vector.tensor_tensor_scan(
                            SC[:, 1 : LK + 1], a_rev, a_rev, 1.0,
                            op0=mybir.AluOpType.mult, op1=mybir.AluOpType.bypass,
                        )
                else:
                    g32 = gpool.tile([B, nh, Dd], F32, tag="g32")
                    SC = gpool.tile([B, LK + 1], F32, tag="SC")
                    nc.vector.memset(g32[:], 0.5)
                    nc.vector.memset(SC[:], 0.5)
                    pg = g32[:, 0, 0:LK] if nh == 1 else g32[:, 0 : LK // Dd, :]

                # per t: diag_t = (eye * pg_t) * suffix_t in one fused
                # tensor_scalar (two per-partition scalars); PE accumulates
                # psum[:, ui, :] += diag_t @ x_t
                for t in range(0 if skip_mm else LK):
                    dg = dpool.tile([128, 128], x_dt, tag="dg")
                    if nh == 1:
                        pg_t = pg[:, t : t + 1]
                    else:
                        pg_t = pg[:, t // Dd, t % Dd : t % Dd + 1]
                    if scan_v2 and t == LK - 1:
                        nc.vector.tensor_scalar(
                            dg[:], eye[:], pg_t, None, op0=mybir.AluOpType.mult
                        )
                    else:
                        sc_t = (
                            SC[:, LK - 2 - t : LK - 1 - t]
                            if scan_v2
                            else SC[:, LK - 1 - t : LK - t]
                        )
                        nc.vector.tensor_scalar(
                            dg[:], eye[:], pg_t, sc_t,
                            op0=mybir.AluOpType.mult, op1=mybir.AluOpType.mult,
                        )
                    nc.tensor.matmul(
                        ps_g[:, ui, :], dg[:], xg[:, t, :],
                        start=(t == 0), stop=(t == LK - 1),
                        skip_group_check=True,
                    )

                if u % G == G - 1:
                    # one eviction + one output DMA per group of G instances;
                    # eviction on ACT (its func table is resident from the
                    # cast-copy; DVE is the busiest engine)
                    if skip_mm:
                        nc.vector.memset(out_g[:], 0.0)
                    elif x_split:
                        nc.scalar.copy(out_g[:], ps_g[:])
                    else:
                        nc.vector.tensor_copy(out_g[:], ps_g[:])
                    if not skip_out:
                        eng = nc.sync if out_ring == "sync" else nc.gpsimd
                        eng.dma_start(out_dram[:], out_g[:])

            if loop_k is None:
                body()
            else:
                assert loop_k % unroll == 0, (loop_k, unroll)
                assert unroll % G == 0, (unroll, G)
                with tc.For_i(0, loop_k // unroll, 1) as iv:
                    for _u in range(unroll):
                        body(_u)

    nc.compile()
    return nc


def get_nc(loop_k=None, fallback=False, overrides=None):
    cfg = dict(CONFIG)
    if overrides:
        cfg.update(overrides)
    if fallback:
        # conservative: keep 128 steps in fp32 (truncation rel-err < 1e-55
        # for uniform gates)
        cfg.update(LK=128, x_bf16=False, x_bufs=2)
    key = (loop_k, tuple(sorted(cfg.items())))
    if key not in _NC_CACHE:
        _NC_CACHE[key] = _build_nc(loop_k=loop_k, **cfg)
    return _NC_CACHE[key]


def _bf16_round(a):
    """Round fp32 array to bf16-representable fp32 values (ties-to-even)."""
    u = a.astype(np.float32).view(np.uint32)
    r = (u + 0x7FFF + ((u >> 16) & 1)) & 0xFFFF0000
    return r.astype(np.uint32).view(np.float32)


def make_in_maps(x, push_gate, pop_gate, lk=None):
    if lk is None:
        lk = CONFIG["LK"]
    x = np.asarray(x, dtype=np.float32)
    pg = np.asarray(push_gate, dtype=np.float32).reshape(B_TOTAL, L)[:, L - lk :]
    og = np.asarray(pop_gate, dtype=np.float32).reshape(B_TOTAL, L)[:, L - lk :]
    g = np.concatenate([pg, og], axis=1).astype(np.float32)  # [B, 2lk]
    hi = _bf16_round(g)
    lo = _bf16_round(g - hi)
    nh = -(-2 * lk // D)
    grow = np.zeros((B_TOTAL, 2 * nh, D), np.float32)
    grow.reshape(B_TOTAL, -1)[:, 0 : 2 * lk] = hi
    grow.reshape(B_TOTAL, -1)[:, nh * D : nh * D + 2 * lk] = lo
    xg = np.concatenate([x[:, L - lk :, :], grow], axis=1)  # [B, lk+2nh, D]
    if CONFIG.get("flat_dram", False):
        xg = xg.reshape(B_TOTAL, -1)
    xg = np.ascontiguousarray(xg)
    return [{"xg": xg[c * B_LOC : (c + 1) * B_LOC]} for c in range(N_CORES)]


def assemble_out(results):
    # single-shot out is [B_LOC, 1, D]; loop builds are [B_LOC, G, D] with
    # every group slot holding the same instance result
    return np.concatenate(
        [
            np.asarray(results[c]["out"]).reshape(B_LOC, -1, D)[:, -1, :]
            for c in range(N_CORES)
        ],
        axis=0,
    )


def _truncation_safe(og_2d, lk):
    """True if dropping timesteps t < L-lk keeps us far inside the 2e-2 gate.

    Every dropped term's weight is bounded by prod_{s in kept range}(1-o_s);
    the dropped state h has O(1) rows, so bounding the max and rms kept-range
    suffix products bounds the truncation rel-err.
    """
    tail = 1.0 - og_2d[:, L - lk :].astype(np.float64)
    with np.errstate(divide="ignore"):
        lg = np.log(np.maximum(tail, 0.0))
    sp = np.exp(lg.sum(axis=1))
    return float(sp.max()) < SP_MAX_THRESHOLD and float(
        np.sqrt((sp**2).mean())
    ) < SP_RMS_THRESHOLD


def kernel(x, push_gate, pop_gate):
    from concourse.bass_utils import run_bass_kernel_spmd

    x = np.asarray(x, dtype=np.float32)
    pg = np.asarray(push_gate, dtype=np.float32)
    og = np.asarray(pop_gate, dtype=np.float32)

    lk = CONFIG["LK"]
    if _truncation_safe(og.reshape(B_TOTAL, L), lk):
        nc = get_nc()
        in_maps = make_in_maps(x, pg, og, lk)
    else:
        # pathological gates: conservative long-window fp32 variant
        nc = get_nc(fallback=True)
        in_maps = make_in_maps(x, pg, og, 128)
    res = run_bass_kernel_spmd(nc, in_maps, list(range(N_CORES)))
    return assemble_out(res.results).astype(np.float32)
